# revision 29
# baseline (speedup 1.0000x reference)
"""CNN+SE+LSTM fused Trainium2 kernel (v3).

Data-parallel over batch: B=2048 split across 8 NeuronCores (256 each).

Highlights:
  - conv1x1 in fp8(e4m3) DoubleRow perf mode: 2x128 input channels per
    matmul at 0.5 cyc/col. x is cast + pre-tiled to the exact per-group
    SBUF layout on the host (contiguous 256 KiB DMAs, 4x less traffic).
  - groups processed in pairs: each sigmoid covers [128, 1024] across 2
    PSUM banks (same conv-bias chunk), halving ACT instruction bubbles.
  - SE softmax exp = 5th-order Horner polynomial on DVE (|z| < ~1), so
    ACT never swaps its activation table (sigmoid/tanh share one set).
  - maxpool over the window = max-tree: level 1 on GpSimd, levels 2+ on
    DVE 2x-mode tensor_tensor (~3x cheaper than one 64-deep TensorReduce).
  - LSTM (bf16, single step, dead forget gate) restructured: the 4
    q-chunks of each gate share one PSUM tile, biases enter as rank-1
    matmuls, one activation per (direction, gate) over [128, 512].
    The batch is split in 2 halves; half 0's LSTM+classifier interleaves
    with the last conv blocks, so only half 1 runs in the tail.
  - LSTM weight loads are chunked onto the two HWDGE rings between the
    early blocks (SWDGE keeps the latency-critical sebc broadcasts).
"""

import numpy as np

import concourse.bass as bass
import concourse.tile as tile
from concourse import bacc, mybir
from concourse.bass_utils import run_bass_kernel_spmd

B, W, D, U, H = 2048, 64, 512, 512, 512
NC = 8
BS = B // NC          # 256 batch rows per core
GB = 8                # batches per group (8 * W = 512 matmul columns)
NG = BS // GB         # 32 groups
BLOCKS = [4, 4, 4, 4, 4, 4, 4, 4]   # SE batching (pairs of groups)
assert sum(BLOCKS) == NG
DC = D // 128         # 4 contraction chunks
UC = U // 128         # 4 output-channel chunks
BSH = BS // 2         # batch half for the LSTM

dt = mybir.dt
AF = mybir.ActivationFunctionType
ALU = mybir.AluOpType
AX = mybir.AxisListType
DR = mybir.MatmulPerfMode.DoubleRow

import os
CONV_MODE = os.environ.get("CONV_MODE", "dr")
_STATE = None


def _build_bass(unroll=1):
    nc = bacc.Bacc("TRN2", target_bir_lowering=False, debug=False,
                   num_devices=NC, num_swdge_queues=4)

    f32, f32r, bf16, fp8 = dt.float32, dt.float32r, dt.bfloat16, dt.float8e4

    xdt = bf16 if CONV_MODE == "bf16" else fp8
    # x pre-arranged on host to per-group tile layout: [g][p][dc][(b w)] so
    # each group's DMA is one contiguous read (2 KiB/partition in fp8).
    d_xt = nc.dram_tensor("xt", [NG, 128, DC * GB * W], xdt,
                          kind="ExternalInput").ap()
    d_cw = nc.dram_tensor("cw", [128, DC, U], xdt, kind="ExternalInput").ap()
    d_cb = nc.dram_tensor("cb", [128, UC], f32, kind="ExternalInput").ap()
    d_ones = nc.dram_tensor("onescol", [128, 1], bf16, kind="ExternalInput").ap()
    d_onesrow = nc.dram_tensor("onesrow", [1, BS], bf16, kind="ExternalInput").ap()
    d_ones32 = nc.dram_tensor("ones32", [1, 4 * GB], f32r, kind="ExternalInput").ap()
    d_sewt = nc.dram_tensor("sewt", [W, W], f32r, kind="ExternalInput").ap()
    d_seb = nc.dram_tensor("seb", [1, W], f32r, kind="ExternalInput").ap()
    d_w0, d_b0, d_w1, d_b1 = {}, {}, {}, {}
    for s in ("f", "r"):
        d_w0[s] = nc.dram_tensor(f"w0{s}", [128, 4, 1536], bf16, kind="ExternalInput").ap()
        d_b0[s] = nc.dram_tensor(f"b0{s}", [1, 1536], bf16, kind="ExternalInput").ap()
        d_w1[s] = nc.dram_tensor(f"w1{s}", [128, 8, 1536], bf16, kind="ExternalInput").ap()
        d_b1[s] = nc.dram_tensor(f"b1{s}", [1, 1536], bf16, kind="ExternalInput").ap()
    d_clsw = nc.dram_tensor("clsw", [128, 8], bf16, kind="ExternalInput").ap()
    d_clsb = nc.dram_tensor("clsb", [1, 1], f32, kind="ExternalInput").ap()
    d_out = nc.dram_tensor("out", [1, BS], f32, kind="ExternalOutput").ap()

    with tile.TileContext(nc) as tc:
        with tc.tile_pool(name="wpool", bufs=1) as wpool, \
             tc.tile_pool(name="persist", bufs=1) as persist:
            # small static weights, staged up front on the SWDGE path
            cw_t = wpool.tile([128, DC, U], xdt, name="cw_t")
            nc.gpsimd.dma_start(cw_t[:], d_cw)
            cb_t = wpool.tile([128, UC], f32, name="cb_t")
            nc.gpsimd.dma_start(cb_t[:], d_cb)
            ones_t = wpool.tile([128, 1], bf16, name="ones_t")
            nc.gpsimd.dma_start(ones_t[:], d_ones)
            onesrow_t = wpool.tile([1, BS], bf16, name="onesrow_t")
            nc.gpsimd.dma_start(onesrow_t[:], d_onesrow)
            ones32_t = wpool.tile([1, 4 * GB], f32r, name="ones32_t")
            nc.gpsimd.dma_start(ones32_t[:], d_ones32)
            sewt_t = wpool.tile([W, W], f32r, name="sewt_t")
            nc.gpsimd.dma_start(sewt_t[:], d_sewt)
            seb_t = wpool.tile([1, W], f32r, name="seb_t")
            nc.gpsimd.dma_start(seb_t[:], d_seb)
            w0_t, b0_t, w1_t, b1_t = {}, {}, {}, {}
            for s in ("f", "r"):
                w0_t[s] = wpool.tile([128, 4, 1536], bf16, name=f"w0{s}_t")
                w1_t[s] = wpool.tile([128, 8, 1536], bf16, name=f"w1{s}_t")
                b0_t[s] = wpool.tile([1, 1536], bf16, name=f"b0{s}_t")
                nc.gpsimd.dma_start(b0_t[s][:], d_b0[s])
                b1_t[s] = wpool.tile([1, 1536], bf16, name=f"b1{s}_t")
                nc.gpsimd.dma_start(b1_t[s][:], d_b1[s])
            clsw_t = wpool.tile([128, 8], bf16, name="clsw_t")
            nc.gpsimd.dma_start(clsw_t[:], d_clsw)
            clsb_t = wpool.tile([1, 1], f32, name="clsb_t")
            nc.gpsimd.dma_start(clsb_t[:], d_clsb)

            # pooled^T accumulator [128, uc, BS], filled per group
            pooledT = persist.tile([128, UC, BS], bf16, name="pooledT")

            for _rep in range(unroll):
                with tc.tile_pool(name="xp", bufs=3) as xp, \
                     tc.tile_pool(name="sigp", bufs=5) as sigp, \
                     tc.tile_pool(name="scp", bufs=2) as scp, \
                     tc.tile_pool(name="bcp", bufs=2) as bcp, \
                     tc.tile_pool(name="sep", bufs=3) as sep, \
                     tc.tile_pool(name="lp", bufs=2) as lp, \
                     tc.tile_pool(name="op", bufs=1) as op, \
                     tc.tile_pool(name="drp", bufs=4, space="DRAM") as drp, \
                     tc.tile_pool(name="cps", bufs=2, space="PSUM") as cps, \
                     tc.tile_pool(name="usps", bufs=1, space="PSUM") as usps, \
                     tc.tile_pool(name="lgps", bufs=1, space="PSUM") as lgps, \
                     tc.tile_pool(name="gps", bufs=2, space="PSUM") as gps:

                    # ---------- LSTM half (bf16, batched gates) ----------
                    def lstm_dir(w_t, b_t, kcs, rhs, out4, out_tanh, h0):
                        cols = onesrow_t[0:1, h0:h0 + BSH]
                        gate = {}
                        for gi, func in ((0, AF.Sigmoid), (1, AF.Tanh),
                                         (2, AF.Sigmoid)):
                            gp4 = gps.tile([128, 4, BSH], f32, name="gp4", tag="gp4")
                            for q in range(4):
                                m = gi * 4 + q
                                for kc in range(kcs):
                                    nc.tensor.matmul(
                                        gp4[:, q, :],
                                        w_t[:, kc, m * 128:(m + 1) * 128],
                                        rhs[kc],
                                        start=(kc == 0), stop=False,
                                    )
                                nc.tensor.matmul(
                                    gp4[:, q, :],
                                    b_t[0:1, m * 128:(m + 1) * 128],
                                    cols,
                                    start=False, stop=True,
                                )
                            gsb = lp.tile([128, 4, BSH], f32,
                                          name="gsb", tag=f"gate{gi}")
                            nc.scalar.activation(
                                gsb[:].rearrange("p q b -> p (q b)"),
                                gp4[:].rearrange("p q b -> p (q b)"), func)
                            gate[gi] = gsb
                        cpre = lp.tile([128, 4, BSH], f32, name="cpre", tag="cpre")
                        nc.vector.tensor_mul(cpre[:], gate[0][:], gate[1][:])
                        tcl = lp.tile([128, 4, BSH], f32, name="tcl", tag="tcl")
                        nc.scalar.activation(
                            tcl[:].rearrange("p q b -> p (q b)"),
                            cpre[:].rearrange("p q b -> p (q b)"), AF.Tanh)
                        if out_tanh:
                            h = lp.tile([128, 4, BSH], f32, name="h", tag="h")
                            nc.vector.tensor_mul(h[:], gate[2][:], tcl[:])
                            nc.scalar.activation(
                                out4[:].rearrange("p q b -> p (q b)"),
                                h[:].rearrange("p q b -> p (q b)"), AF.Tanh)
                        else:
                            nc.vector.tensor_mul(out4[:], gate[2][:], tcl[:])

                    def emit_lstm_half(hi, stage):
                        h0 = hi * BSH
                        if stage == 0:
                            for s in ("f", "r"):
                                o0 = op.tile([128, 4, BSH], bf16, name=f"o0{s}{hi}")
                                pooled_rhs = [pooledT[:, kc, h0:h0 + BSH]
                                              for kc in range(UC)]
                                lstm_dir(w0_t[s], b0_t[s], 4, pooled_rhs,
                                         o0, False, h0)
                                setattr(emit_lstm_half, f"o0{s}{hi}", o0)
                            return
                        o0f = getattr(emit_lstm_half, f"o0f{hi}")
                        o0r = getattr(emit_lstm_half, f"o0r{hi}")
                        o0_rhs = [o0f[:, q, :] for q in range(4)] + \
                                 [o0r[:, q, :] for q in range(4)]
                        o1 = {}
                        for s in ("f", "r"):
                            o1[s] = op.tile([128, 4, BSH], bf16, name=f"o1{s}{hi}")
                            lstm_dir(w1_t[s], b1_t[s], 8, o0_rhs, o1[s], True, h0)
                        clsp = gps.tile([1, BSH], f32, name="clsp", tag="gp4")
                        for kc in range(8):
                            nc.tensor.matmul(
                                clsp[:], clsw_t[:, kc:kc + 1],
                                o1["f" if kc < 4 else "r"][:, kc % 4, :],
                                start=(kc == 0), stop=(kc == 7),
                            )
                        outsb = lp.tile([1, BSH], f32, name="outsb", tag="outsb")
                        nc.scalar.activation(
                            outsb[:], clsp[:], AF.Tanh,
                            bias=clsb_t[0:1, 0:1], scale=1.0)
                        nc.sync.dma_start(d_out[0:1, h0:h0 + BSH], outsb[:])

                    # Chunked prefetch of the big LSTM weights on the two
                    # HWDGE rings at early block boundaries.
                    pf = []
                    for wt, dw in ((w0_t["f"], d_w0["f"]), (w0_t["r"], d_w0["r"]),
                                   (w1_t["f"], d_w1["f"]), (w1_t["r"], d_w1["r"])):
                        half = 768
                        pf.append((wt[:, :, 0:half], dw[:, :, 0:half]))
                        pf.append((wt[:, :, half:], dw[:, :, half:]))

                    # ---------------- phase 1: conv + SE + maxpool ----------------
                    # One iteration per PAIR of groups; the SE softmax runs
                    # per pair too, so its ~6us latency chain (us -> DRAM
                    # transpose bounce -> lg matmul -> exp poly -> softmax ->
                    # scr2 -> sebc broadcast) pipelines across pairs instead
                    # of serializing once per 4-group block.
                    NB = 2 * GB            # 16 rows per SE batch
                    pending = None
                    for p in range(NG // 2):
                        pair = (2 * p, 2 * p + 1)
                        if 2 <= p <= 9:
                            dst, src_ = pf[p - 2]
                            (nc.sync if p % 2 == 0 else nc.scalar).dma_start(dst, src_)
                        xt = xp.tile([128, 2, DC, GB * W], xdt, name="xt", tag="x")
                        for gi, g in enumerate(pair):
                            dma_eng = nc.sync if g % 2 == 0 else nc.scalar
                            dma_eng.dma_start(
                                xt[:, gi].rearrange("p dc w -> p (dc w)"),
                                d_xt[g])

                        sigg = sigp.tile([128, UC, 2, GB * W], bf16,
                                         name="sigg", tag="sig")
                        for uc in range(UC):
                            cp = cps.tile([128, 2, GB * W], f32, name="cp", tag="cp")
                            for gi in range(2):
                                if CONV_MODE == "dr":
                                    for i in range(2):
                                        nc.tensor.matmul(
                                            cp[:, gi, :],
                                            cw_t[:, 2 * i:2 * i + 2,
                                                 uc * 128:(uc + 1) * 128],
                                            xt[:, gi, 2 * i:2 * i + 2, :],
                                            start=(i == 0),
                                            stop=(i == 1),
                                            perf_mode=DR,
                                        )
                                else:
                                    for i in range(DC):
                                        nc.tensor.matmul(
                                            cp[:, gi, :],
                                            cw_t[:, i, uc * 128:(uc + 1) * 128],
                                            xt[:, gi, i, :],
                                            start=(i == 0),
                                            stop=(i == DC - 1),
                                        )
                            nc.scalar.activation(
                                sigg[:, uc].rearrange("p g w -> p (g w)"),
                                cp[:].rearrange("p g w -> p (g w)"),
                                AF.Sigmoid,
                                bias=cb_t[:, uc:uc + 1], scale=1.0,
                            )
                        # channel mean via ones-matmul
                        scr1 = drp.tile([2, GB * W], f32r, name="scr1", tag="scr1")
                        for gi, g in enumerate(pair):
                            us = usps.tile([1, GB * W], f32, name="us", tag="us")
                            for uc in range(UC):
                                nc.tensor.matmul(
                                    us[:], ones_t[:], sigg[:, uc, gi, :],
                                    start=(uc == 0), stop=(uc == UC - 1),
                                )
                            # PSUM -> SBUF bounce (DMA can't read PSUM);
                            # on ACT: DVE is the phase-1 bottleneck engine
                            avg_row = sep.tile([1, GB * W], f32r,
                                               name="avg_row", tag="avgrow")
                            nc.scalar.copy(avg_row[:].bitcast(f32), us[:])
                            nc.sync.dma_start(scr1[gi:gi + 1, :], avg_row[:])

                        # SE for the pair: avgT [w, (gi b)]
                        avgT = sep.tile([W, NB], f32r, name="avgT", tag="avgT")
                        nc.sync.dma_start(
                            avgT[:],
                            scr1[:].rearrange("g (b w) -> (w) g b", w=W),
                        )
                        lg = lgps.tile([NB, W], f32, name="lg", tag="lg")
                        nc.tensor.matmul(lg[:], avgT[:], sewt_t[:],
                                         start=True, stop=False)
                        nc.tensor.matmul(lg[:], ones32_t[:, 0:NB], seb_t[:],
                                         start=False, stop=True)
                        # exp(z) ~ 1+z(1+z/2(1+z/3(1+z/4(1+z/5)))) on DVE: |z|<~1
                        # so the 5th-order tail is <1e-3 relative; ACT keeps its
                        # sigmoid/tanh table loaded the whole kernel.
                        zs = sep.tile([NB, W], f32, name="zs", tag="zs")
                        nc.vector.tensor_copy(zs[:], lg[:])
                        E = sep.tile([NB, W], f32, name="E", tag="E")
                        hp = sep.tile([NB, W], f32, name="hp", tag="hp")
                        hq = sep.tile([NB, W], f32, name="hq", tag="hq")
                        nc.vector.tensor_scalar(
                            hp[:], zs[:], 1.0 / 5, 1.0, op0=ALU.mult, op1=ALU.add)
                        for k in (4, 3, 2):
                            nc.vector.tensor_mul(hq[:], zs[:], hp[:])
                            nc.vector.tensor_scalar(
                                hp[:], hq[:], 1.0 / k, 1.0, op0=ALU.mult, op1=ALU.add)
                        nc.vector.tensor_mul(hq[:], zs[:], hp[:])
                        nc.vector.tensor_scalar(
                            E[:], hq[:], 1.0, 1.0, op0=ALU.mult, op1=ALU.add)
                        S = sep.tile([NB, 1], f32, name="S", tag="S")
                        nc.vector.reduce_sum(S[:], E[:], axis=AX.X)
                        R = sep.tile([NB, 1], f32, name="R", tag="R")
                        nc.vector.reciprocal(R[:], S[:])
                        seg = sep.tile([NB, W], f32r, name="seg", tag="seg")
                        nc.vector.tensor_scalar_mul(seg[:], E[:], R[:, 0:1])
                        scr2 = drp.tile([NB, W], f32r, name="scr2", tag="scr2")
                        nc.scalar.dma_start(scr2[:], seg[:])
                        # broadcast to all partitions with f32r->bf16 cast (SWDGE)
                        sebc = bcp.tile([128, NB * W], bf16, name="sebc", tag="sebc")
                        nc.gpsimd.dma_start(
                            sebc[:],
                            scr2[:].bitcast(f32)
                            .rearrange("b w -> (b w)").unsqueeze(0)
                            .broadcast_to([128, NB * W]),
                        )
                        def emit_scale_max(pair, sigg, sebc):
                          for gi, g in enumerate(pair):
                            scaled = scp.tile([128, UC, GB * W], bf16,
                                              name="scaled", tag="scaled")
                            nc.vector.tensor_mul(
                                scaled[:],
                                sigg[:, :, gi, :],
                                sebc[:, gi * GB * W:(gi + 1) * GB * W]
                                .unsqueeze(1).broadcast_to([128, UC, GB * W]),
                            )
                            # max over w=64 as a max-tree on DVE 2x-mode
                            # tensor_tensor + final 8-way TensorReduce: ~1.5x
                            # cheaper than one 64-deep reduce (no fast path).
                            # (gpsimd can't run TensorTensor: walrus rejects
                            # the opcode on the Pool engine.)
                            sv = scaled[:].rearrange("p u (b w) -> p (u b) w", w=W)
                            t1 = scp.tile([128, UC * GB, 32], bf16, name="t1", tag="t1")
                            nc.vector.tensor_tensor(
                                t1[:], sv[:, :, 0:32], sv[:, :, 32:64], op=ALU.max)
                            t2 = scp.tile([128, UC * GB, 16], bf16, name="t2", tag="t2")
                            nc.vector.tensor_tensor(
                                t2[:], t1[:, :, 0:16], t1[:, :, 16:32], op=ALU.max)
                            t3 = scp.tile([128, UC * GB, 8], bf16, name="t3", tag="t3")
                            nc.vector.tensor_tensor(
                                t3[:], t2[:, :, 0:8], t2[:, :, 8:16], op=ALU.max)
                            pbf = scp.tile([128, UC * GB], bf16, name="pbf", tag="pbf")
                            nc.vector.tensor_reduce(
                                pbf[:], t3[:], axis=AX.X, op=ALU.max)
                            nc.vector.tensor_copy(
                                pooledT[:, :, g * GB:(g + 1) * GB],
                                pbf[:].rearrange("p (u b) -> p u b", u=UC),
                            )

                        # one-pair software pipeline: the scale/max of the
                        # PREVIOUS pair runs now (its sebc landed during this
                        # pair's conv), so the in-order DVE queue never waits
                        # on the current pair's sebc round-trip.
                        if pending is not None:
                            emit_scale_max(*pending)
                        pending = (pair, sigg, sebc)

                        # interleave LSTM half 0 with the tail pairs; emitted
                        # late enough that its inputs are ready when the
                        # in-order engine queues reach it.
                        if p == 11:
                            emit_lstm_half(0, stage=0)
                        elif p == 13:
                            emit_lstm_half(0, stage=1)
                    emit_scale_max(*pending)

                    # ---------------- phase 2 tail: LSTM half 1 ----------------
                    emit_lstm_half(1, stage=0)
                    emit_lstm_half(1, stage=1)

    nc.compile()
    return nc


def _prep_weights(i):
    """Host-side packing of the replicated (non-batch) tensors."""
    import ml_dtypes

    fp8 = ml_dtypes.float8_e4m3
    bf16 = ml_dtypes.bfloat16

    def f32(a):
        return np.ascontiguousarray(a, dtype=np.float32)

    out = {}
    xdt = bf16 if CONV_MODE == "bf16" else fp8
    out["cw"] = np.ascontiguousarray(
        f32(i["conv_w"]).T.reshape(DC, 128, U).transpose(1, 0, 2), dtype=xdt
    )
    out["cb"] = f32(i["conv_b"].reshape(UC, 128).T)
    out["onescol"] = np.full((128, 1), 1.0 / U, bf16)
    out["onesrow"] = np.ones((1, BS), bf16)
    out["ones32"] = np.ones((1, 4 * GB), np.float32)
    out["sewt"] = f32(i["se_w"].T)
    out["seb"] = f32(i["se_b"].reshape(1, W))
    igo = np.r_[0:512, 1024:2048]  # drop dead forget gate
    for lvl, kin in (("0", 512), ("1", 1024)):
        for s in ("f", "r"):
            tag = f"l{lvl}{s}"
            wT = f32(i[f"w_ih_{tag}"]).T[:, igo]              # [kin, 1536]
            out[f"w{lvl}{s}"] = np.ascontiguousarray(
                wT.reshape(kin // 128, 128, 1536).transpose(1, 0, 2), dtype=bf16)
            bs = (f32(i[f"b_ih_{tag}"]) + f32(i[f"b_hh_{tag}"]))[igo]
            out[f"b{lvl}{s}"] = np.ascontiguousarray(bs.reshape(1, 1536), dtype=bf16)
    out["clsw"] = np.ascontiguousarray(
        f32(i["cls_w"].reshape(2 * H)).reshape(8, 128).T, dtype=bf16)
    out["clsb"] = f32(i["cls_b"]).reshape(1, 1)
    return out


def _get_nc():
    global _STATE
    if _STATE is None:
        _STATE = _build_bass()
    return _STATE


def make_in_maps(**inputs):
    import ml_dtypes

    w = _prep_weights(inputs)
    # [B, W, D] -> per-core [NG, 128, DC*(GB*W)] fp8 with d = dc*128 + p.
    xdt = ml_dtypes.bfloat16 if CONV_MODE == "bf16" else ml_dtypes.float8_e4m3
    x = np.asarray(inputs["x"], dtype=np.float32)
    xt = x.transpose(2, 0, 1).astype(xdt)                         # [D, B, W]
    maps = []
    for c in range(NC):
        xc = xt[:, c * BS:(c + 1) * BS, :]                        # [D, BS, W]
        x8 = xc.reshape(DC, 128, NG, GB, W).transpose(2, 1, 0, 3, 4)
        m = dict(w)
        m["xt"] = np.ascontiguousarray(x8.reshape(NG, 128, DC * GB * W))
        maps.append(m)
    return maps


def kernel(**inputs):
    nc = _get_nc()
    maps = make_in_maps(**inputs)
    res = run_bass_kernel_spmd(nc, maps, core_ids=list(range(NC)))
    out = np.empty((B, 1), np.float32)
    for c in range(NC):
        out[c * BS:(c + 1) * BS, 0] = res.results[c]["out"][0]
    return out
